# revision 27
# baseline (speedup 1.0000x reference)
"""Trainium2 Bass kernel for 16-head MHA (d_model=1024, batch 4, seq 2048).

Sharding: batch (4) x head-group (2) across 8 NeuronCores. Each core computes
one batch sample's attention for 8 of the 16 heads plus its partial output
projection; the host sums the two partial outputs per sample and adds the
bias terms.

v2 design (ACT-paced): the softmax exp stream on the scalar engine is the
kernel's critical resource (33.5M exps ~ 1.15us per [128,1024] ACTIVATE).
The loop is organised so ACT never starves:

  - Per head-pair hp, queries are processed in four 512-wide blocks (qq).
    Per key chunk kc (128 keys): the two heads' score matmuls (K=64) are
    issued back-to-back at tile positions (0,0)/(64,0) so they execute
    CONCURRENTLY in disjoint row-groups of the PE array (2x S1 throughput),
    writing one [128, 1024] PSUM tile; ONE exp covers both heads.
  - S2 (att@V, M=65 with fused ones-column rowsums) trails exp by 2 chunks.
  - All projection work (q/k for later head pairs, v chunks, y output) is
    chopped into ~0.5us "filler" units pumped into the PE queue between kc
    iterations, sized to fit the PE slack under the ACT period, so the PE
    stays busy while ACT crunches and projections never block the exp flow.
  - The softmax exp is split across engines: 2/3 on ACT (table exp), 1/3 on
    DVE as a Schraudolph bitcast-exp (one tensor_scalar: i16 = round(s *
    128/ln2 + 16250.5), whose bit pattern is bf16 exp(s); adds ~2e-3 rel
    err, total 4.3e-3 vs the 2e-2 budget).
  - Input DMA is split across two software-DGE queues in exact consume
    order (wk, xk halves, wq, xq halves, wo on the gpsimd queue; biases,
    wv, xv chunks on the scalar queue; y output on gpsimd). The slow sync
    (hardware-DGE) queue is avoided entirely.

PSUM budget (8 banks): s1 2x[128,1024] = 4, acc 2x[65,512] = 2, pj 2x[128,512] = 2.

fp16 is used for the whole q/k/scores path: bf16's 8-bit mantissa gives score
errors ~0.05 which exp() amplifies to ~2e-2 output error; fp16 keeps it ~3e-3.
"""

from collections import deque
from contextlib import ExitStack

import numpy as np

import concourse.bacc as bacc
import concourse.mybir as mybir
import concourse.tile as tile
from concourse.bass_utils import run_bass_kernel_spmd

F32 = mybir.dt.float32
F16 = mybir.dt.float16
BF16 = mybir.dt.bfloat16

D = 1024          # d_model
HD = 64           # head dim
NH_CORE = 8       # heads per core
OC = NH_CORE * HD # per-core q/k/v output dims (512)
N_CORES = 8
NI = D // 128     # contraction chunks for projections
NOC = OC // 128   # o-chunks (head pairs)
NDC = D // 128    # output-dim chunks for the final projection


def build_kernel(S=2048):
    nc = bacc.Bacc("TRN2", target_bir_lowering=False, debug=False)

    xq_d = nc.dram_tensor("xqT", (D, S), F16, kind="ExternalInput")
    xk_d = nc.dram_tensor("xkT", (D, S), F16, kind="ExternalInput")
    xv_d = nc.dram_tensor("xvT", (D, S), F16, kind="ExternalInput")
    wq_d = nc.dram_tensor("WqT", (D, OC), F16, kind="ExternalInput")
    wk_d = nc.dram_tensor("WkT", (D, OC), F16, kind="ExternalInput")
    wv_d = nc.dram_tensor("WvT", (D, OC), F16, kind="ExternalInput")
    wo_d = nc.dram_tensor("WoT", (OC, D), F16, kind="ExternalInput")
    bq_d = nc.dram_tensor("bq", (OC,), F32, kind="ExternalInput")
    bk_d = nc.dram_tensor("bk", (OC,), F32, kind="ExternalInput")
    y_d = nc.dram_tensor("yT", (D, S), F32, kind="ExternalOutput")

    NKC = S // 128        # key chunks (16)
    QQ = 512              # query block
    NQQ = S // QQ         # 4
    PEND = 2              # S2 trails exp by this many chunks
    VT = 512              # xv chunk width (keys)
    NVC = S // VT         # xv chunks (4)

    Exp = mybir.ActivationFunctionType.Exp
    Mult = mybir.AluOpType.mult
    Add = mybir.AluOpType.add
    I16 = mybir.dt.int16
    # Schraudolph exp in bf16-bits: i16 = round(s * 128/ln2 + (127*128 - C));
    # the int16 bit pattern IS the bf16 approximation of exp(s). Scores stay
    # within |s| < ~50 so i16 is always in [7e3, 26e3] — no wrap/saturation.
    # Empirical rel-err of the full MHA with this exp: 6.3e-3 (budget 2e-2).
    SCH_A = 128.0 / np.log(2.0)
    SCH_B = 127.0 * 128.0 - 5.5

    def exp_on_dve(kc):
        # 1/3 of exps on DVE, 2/3 on ACT: balances the two engines given
        # DVE's other element-wise load (measured: DVE ~1.23us vs ACT
        # ~1.11us per [128,1024] exp)
        return kc % 3 == 2

    with tile.TileContext(nc) as tc, ExitStack() as ctx:
        wpool = ctx.enter_context(tc.tile_pool(name="w", bufs=1))
        xpool = ctx.enter_context(tc.tile_pool(name="x", bufs=1))
        xvpool = ctx.enter_context(tc.tile_pool(name="xv", bufs=2))
        spool = ctx.enter_context(tc.tile_pool(name="seq", bufs=1))
        epool = ctx.enter_context(tc.tile_pool(name="e", bufs=6))
        npool = ctx.enter_context(tc.tile_pool(name="nrm", bufs=2))
        evpool = ctx.enter_context(tc.tile_pool(name="ev", bufs=2))
        pjpool = ctx.enter_context(tc.tile_pool(name="pj", bufs=2, space="PSUM"))
        s1pool = ctx.enter_context(tc.tile_pool(name="s1", bufs=2, space="PSUM"))
        accpool = ctx.enter_context(tc.tile_pool(name="acc", bufs=2, space="PSUM"))

        # ---- resident weights / biases / x slabs ----
        wq_sb = wpool.tile([128, NI, OC], F16, tag="wq")
        wk_sb = wpool.tile([128, NI, OC], F16, tag="wk")
        wv_sb = wpool.tile([128, NI, OC], F16, tag="wv")
        wo_sb = wpool.tile([128, NOC, D], F16, tag="wo")
        bq_sb = wpool.tile([128, NOC], F32, tag="bq")
        bk_sb = wpool.tile([128, NOC], F32, tag="bk")
        xq_sb = xpool.tile([128, NI, S], F16, tag="xq")
        xk_sb = xpool.tile([128, NI, S], F16, tag="xk")

        # gpsimd software-DGE queue, consume order: weights then x halves
        # (t-halves give 2KB/partition DMA lines; kT/qT projections start
        # after half 0)
        H = S // 2
        xk_r = xk_d.ap().rearrange("(ic p) t -> p ic t", p=128)
        xq_r = xq_d.ap().rearrange("(ic p) t -> p ic t", p=128)
        nc.gpsimd.dma_start(out=wk_sb, in_=wk_d.ap().rearrange("(ic p) o -> p ic o", p=128))
        nc.gpsimd.dma_start(out=xk_sb[:, :, 0:H], in_=xk_r[:, :, 0:H])
        nc.gpsimd.dma_start(out=wq_sb, in_=wq_d.ap().rearrange("(ic p) o -> p ic o", p=128))
        # only the first query block of xq gates the first exp: load it as
        # its own (small) descriptor so the exp stream starts ~6us sooner
        nc.gpsimd.dma_start(out=xq_sb[:, :, 0:QQ], in_=xq_r[:, :, 0:QQ])
        nc.gpsimd.dma_start(out=xq_sb[:, :, QQ:H], in_=xq_r[:, :, QQ:H])
        nc.gpsimd.dma_start(out=xk_sb[:, :, H:S], in_=xk_r[:, :, H:S])
        nc.gpsimd.dma_start(out=xq_sb[:, :, H:S], in_=xq_r[:, :, H:S])
        nc.gpsimd.dma_start(out=wo_sb, in_=wo_d.ap().rearrange("(oc p) d -> p oc d", p=128))

        # scalar-engine software-DGE queue (parallel to gpsimd's): biases,
        # wv, xv chunks (kc order); ACT is idle this early so the descriptor
        # dispatch is free
        nc.scalar.dma_start(out=bq_sb, in_=bq_d.ap().rearrange("(c p) -> p c", p=128))
        nc.scalar.dma_start(out=bk_sb, in_=bk_d.ap().rearrange("(c p) -> p c", p=128))
        nc.scalar.dma_start(out=wv_sb, in_=wv_d.ap().rearrange("(ic p) o -> p ic o", p=128))
        xv_r = xv_d.ap().rearrange("(ic p) t -> p ic t", p=128)
        xvc_tiles = []
        for c in range(NVC):
            t = xvpool.tile([128, NI, VT], F16, tag="xvc", name=f"xvc{c}")
            nc.scalar.dma_start(out=t, in_=xv_r[:, :, c * VT:(c + 1) * VT])
            xvc_tiles.append(t)

        # ---- per-sequence slabs ----
        v_sb = spool.tile([128, NKC, NH_CORE * (HD + 1)], BF16, tag="v")
        qT_sb = spool.tile([128, NOC, S], F16, tag="qT")
        kT_sb = spool.tile([128, NOC, S], F16, tag="kT")
        att_sb = spool.tile([128, NOC, S], F16, tag="att")

        # ---- filler unit builders (each unit ~<=0.55us of PE) ----
        def qk_units(hp, w_sb, x_sb, b_sb, dst, tq):
            """Project one [128 out-dims, 512 t] block of qT/kT: 4 units."""
            state = {}

            def mk(ic0):
                def fn():
                    if ic0 == 0:
                        state["pps"] = pjpool.tile(
                            [128, QQ], F32, tag="pj",
                            name=f"pj_{dst.name}_{hp}_{tq}_{ic0}")
                    for ic in (ic0, ic0 + 1):
                        nc.tensor.matmul(
                            state["pps"][:, :],
                            w_sb[:, ic, hp * 128:(hp + 1) * 128],
                            x_sb[:, ic, tq * QQ:(tq + 1) * QQ],
                            start=(ic == 0), stop=(ic == NI - 1),
                        )
                    if ic0 == NI - 2:
                        nc.vector.tensor_scalar_add(
                            out=dst[:, hp, tq * QQ:(tq + 1) * QQ],
                            in0=state["pps"][:, :],
                            scalar1=b_sb[:, hp:hp + 1],
                        )
                return fn
            return [mk(i) for i in range(0, NI, 2)]

        def v_units(c):
            """Project xv chunk c (VT keys, all heads) into v_sb: 10 units."""
            xvc = xvc_tiles[c]
            state = {}
            units = []

            def mk(j, ic0):
                def fn():
                    if ic0 == 0:
                        state[j] = pjpool.tile([128, OC], F32, tag="pj",
                                               name=f"vps{c}_{j}")
                    for ic in (ic0, ic0 + 1):
                        nc.tensor.matmul(
                            state[j][:, :],
                            xvc[:, ic, j * 128:(j + 1) * 128],
                            wv_sb[:, ic, :],
                            start=(ic == 0), stop=(ic == NI - 1),
                        )
                return fn

            def mkcopy(j):
                def fn():
                    tci = c * (VT // 128) + j
                    vv = v_sb[:, tci, :].rearrange("p (h c) -> p h c", h=NH_CORE)
                    nc.vector.tensor_copy(
                        out=vv[:, :, 0:HD],
                        in_=state[j][:, :].rearrange("p (h c) -> p h c", c=HD),
                    )
                    nc.vector.memset(vv[:, :, HD:HD + 1], 1.0)
                return fn

            for j in range(VT // 128):
                for ic0 in range(0, NI, 2):
                    units.append(mk(j, ic0))
                units.append(mkcopy(j))
            return units

        def y_units(qq, dc):
            """Output projection block [128 d, 512 q]: 2 MM units + evac."""
            state = {}

            def mk(oc0):
                def fn():
                    if oc0 == 0:
                        state["yps"] = pjpool.tile([128, QQ], F32, tag="pj",
                                                   name=f"yps{qq}_{dc}")
                    for oc in (oc0, oc0 + 1):
                        nc.tensor.matmul(
                            state["yps"][:, :],
                            wo_sb[:, oc, dc * 128:(dc + 1) * 128],
                            att_sb[:, oc, qq * QQ:(qq + 1) * QQ],
                            start=(oc == 0), stop=(oc == NOC - 1),
                        )
                    if oc0 == NOC - 2:
                        y_sb = evpool.tile([128, QQ], F32, tag="yev",
                                           name=f"yev{qq}_{dc}")
                        # PSUM->SBUF evac on ACT (has slack; DVE is loaded)
                        nc.scalar.copy(out=y_sb[:, :], in_=state["yps"][:, :])
                        nc.gpsimd.dma_start(
                            out=y_d.ap()[dc * 128:(dc + 1) * 128,
                                         qq * QQ:(qq + 1) * QQ],
                            in_=y_sb[:, :],
                        )
                return fn
            return [mk(i) for i in range(0, NOC, 2)]

        # ---- static filler schedule per (hp, qq) ----
        sched = {}

        def kq_all(hp):  # q+k projection for head pair hp (all 4 t-blocks)
            u = []
            for tq in range(NQQ):
                u += qk_units(hp, wk_sb, xk_sb, bk_sb, kT_sb, tq)
            for tq in range(NQQ):
                u += qk_units(hp, wq_sb, xq_sb, bq_sb, qT_sb, tq)
            return u

        # hp0/qq0 fillers. Ordering constraints: kT chunks 2/3 are consumed
        # by this window's own S1s at kc>=8/12 (must be fully emitted before
        # those S1s, but their xk-half-1 DMA lands late, so they go after
        # v chunk 1); v chunk c covers keys 512c..512c+511, needed by this
        # window's S2 flush.
        u00 = []
        u00 += v_units(1)
        u00 += qk_units(0, wk_sb, xk_sb, bk_sb, kT_sb, 2)
        u00 += qk_units(0, wk_sb, xk_sb, bk_sb, kT_sb, 3)
        u00 += v_units(2)
        u00 += v_units(3)
        u00 += qk_units(0, wq_sb, xq_sb, bq_sb, qT_sb, 1)
        sched[(0, 0)] = u00
        sched[(0, 1)] = qk_units(0, wq_sb, xq_sb, bq_sb, qT_sb, 2)
        sched[(0, 2)] = qk_units(0, wq_sb, xq_sb, bq_sb, qT_sb, 3)
        u1 = kq_all(1)
        sched[(0, 3)] = u1[:20]
        sched[(1, 0)] = u1[20:]
        u2 = kq_all(2)
        sched[(1, 1)] = u2[:16]
        sched[(1, 2)] = u2[16:]
        u3 = kq_all(3)
        sched[(2, 0)] = u3[:16]
        sched[(2, 1)] = u3[16:]
        for qq in range(3):
            uy = []
            for dc in range(NDC):
                uy += y_units(qq, dc)
            sched[(3, qq + 1)] = uy

        # ---- prologue: kT(hp0) chunks 0-1, v chunk 0, qT(hp0) block 0 ----
        for fn in qk_units(0, wk_sb, xk_sb, bk_sb, kT_sb, 0):
            fn()
        for fn in qk_units(0, wk_sb, xk_sb, bk_sb, kT_sb, 1):
            fn()
        for fn in v_units(0):
            fn()
        for fn in qk_units(0, wq_sb, xq_sb, bq_sb, qT_sb, 0):
            fn()

        # ---- main loop ----
        def normalize(hp, qq, head, acc):
            off = (head % 2) * 64
            asb = npool.tile([65, QQ], F32, tag="accsb",
                             name=f"asb{hp}_{qq}_{head}")
            nc.scalar.copy(out=asb[:, :], in_=acc[:, :])
            rt = npool.tile([1, QQ], F32, tag="rtmp", name=f"rt{hp}_{qq}_{head}")
            nc.vector.tensor_copy(out=rt[:, :], in_=asb[64:65, :])
            nc.vector.reciprocal_approx_fast(out=rt[:, :], in_=rt[:, :])
            bc = npool.tile([64, QQ], F32, tag="bcast", name=f"bc{hp}_{qq}_{head}")
            nc.gpsimd.partition_broadcast(out_ap=bc[:, :], in_ap=rt[:, :])
            nc.vector.tensor_tensor(
                out=att_sb[off:off + 64, hp, qq * QQ:(qq + 1) * QQ],
                in0=asb[0:64, :],
                in1=bc[:, :],
                op=Mult,
            )

        for hp in range(NOC):
            for qq in range(NQQ):
                units = sched.pop((hp, qq), [])
                nu = len(units)
                ui = 0
                accs = [
                    accpool.tile([65, QQ], F32, tag="acc",
                                 name=f"acc{hp}_{qq}_{h}")
                    for h in range(2)
                ]

                def s2(e, kc):
                    for h in range(2):
                        nc.tensor.matmul(
                            accs[h][:, :],
                            v_sb[:, kc, (2 * hp + h) * (HD + 1):
                                 (2 * hp + h + 1) * (HD + 1)],
                            e[:, h * QQ:(h + 1) * QQ],
                            start=(kc == 0), stop=(kc == NKC - 1),
                        )

                pend = deque()
                for kc in range(NKC):
                    # pump fillers: distribute nu units over 16 kc slots
                    tgt = (nu * (kc + 1)) // NKC
                    while ui < tgt:
                        units[ui]()
                        ui += 1
                    s1 = s1pool.tile([128, 2 * QQ], F32, tag="s1",
                                     name=f"s1_{hp}_{qq}_{kc}")
                    for h in range(2):
                        o = h * 64
                        nc.tensor.matmul(
                            s1[:, h * QQ:(h + 1) * QQ],
                            kT_sb[o:o + 64, hp, kc * 128:(kc + 1) * 128],
                            qT_sb[o:o + 64, hp, qq * QQ:(qq + 1) * QQ],
                            start=True, stop=True,
                        )
                    if exp_on_dve(kc):
                        ei = epool.tile([128, 2 * QQ], I16, tag="ei", bufs=3,
                                        name=f"ei{hp}_{qq}_{kc}")
                        nc.vector.tensor_scalar(
                            out=ei[:, :], in0=s1[:, :],
                            scalar1=SCH_A, scalar2=SCH_B,
                            op0=Mult, op1=Add,
                        )
                        eap = ei[:, :].bitcast(BF16)
                    else:
                        e = epool.tile([128, 2 * QQ], BF16, tag="e", bufs=3,
                                       name=f"e{hp}_{qq}_{kc}")
                        nc.scalar.activation(out=e[:, :], in_=s1[:, :], func=Exp)
                        eap = e[:, :]
                    pend.append((eap, kc))
                    if len(pend) > PEND:
                        s2(*pend.popleft())
                while pend:
                    s2(*pend.popleft())
                for h in range(2):
                    normalize(hp, qq, h, accs[h])

        # ---- tail: y projection for the last query block ----
        for dc in range(NDC):
            for fn in y_units(3, dc):
                fn()

    nc.compile()
    return nc


def make_in_maps(query, key, value, Wq, bq, Wk, bk, Wv, bv, Wo, bo):
    """Shard + lay out full inputs for the 8 cores: core = 2*n + g."""
    f16 = np.float16
    N = query.shape[0]
    per_g = {}
    for g in range(2):
        osl = slice(g * OC, (g + 1) * OC)
        per_g[g] = dict(
            WqT=np.ascontiguousarray(Wq[osl, :].T).astype(f16),
            WkT=np.ascontiguousarray(Wk[osl, :].T).astype(f16),
            WvT=np.ascontiguousarray(Wv[osl, :].T).astype(f16),
            WoT=np.ascontiguousarray(Wo[:, osl].T).astype(f16),
            bq=np.ascontiguousarray(bq[osl]).astype(np.float32),
            bk=np.ascontiguousarray(bk[osl]).astype(np.float32),
        )
    in_maps = []
    for n in range(N):
        xqT = np.ascontiguousarray(query[n].T).astype(f16)
        xkT = np.ascontiguousarray(key[n].T).astype(f16)
        xvT = np.ascontiguousarray(value[n].T).astype(f16)
        for g in range(2):
            m = dict(xqT=xqT, xkT=xkT, xvT=xvT)
            m.update(per_g[g])
            in_maps.append(m)
    return in_maps


_BUILT = None


def _get_built():
    global _BUILT
    if _BUILT is None:
        _BUILT = build_kernel(2048)
    return _BUILT


def kernel(query, key, value, Wq, bq, Wk, bk, Wv, bv, Wo, bo, _results=None):
    query = np.asarray(query, np.float32)
    key = np.asarray(key, np.float32)
    value = np.asarray(value, np.float32)
    Wq, bq = np.asarray(Wq, np.float32), np.asarray(bq, np.float32)
    Wk, bk = np.asarray(Wk, np.float32), np.asarray(bk, np.float32)
    Wv, bv = np.asarray(Wv, np.float32), np.asarray(bv, np.float32)
    Wo, bo = np.asarray(Wo, np.float32), np.asarray(bo, np.float32)

    N, S, _ = query.shape
    if _results is None:
        nc = _get_built()
        in_maps = make_in_maps(query, key, value, Wq, bq, Wk, bk, Wv, bv, Wo, bo)
        res = run_bass_kernel_spmd(nc, in_maps, list(range(N_CORES)))
        _results = res.results

    const = bv @ Wo.T + bo  # host-folded bias terms
    out = np.empty((N, S, D), np.float32)
    for n in range(N):
        yT = _results[2 * n]["yT"] + _results[2 * n + 1]["yT"]
        out[n] = yT.T + const
    return out
